# revision 37
# baseline (speedup 1.0000x reference)
"""Trainium2 Bass kernel for nn_Cross_Domain_Class_Alignment.

Reference computation (per sample b):
    mask0[b] = argmin_k || feature_s2t[b,:,r,c] - centroid_target[k] ||^2
    mask1[b] = argmin_k || feature_target[b,:,r,c] - centroid_s2t[k] ||^2
    both nearest-upsampled from (65,129) to (512,1024), int32.

Sharding: data-parallel over batch B=8 across 8 NeuronCores (1 sample/core).
Centroids are replicated.

Per-core dataflow (per mask):
  - features [256, 8385] streamed in 2048-pixel slices x 2 channel chunks
  - dist matmuls, centroid-stationary: psum quad [128, 512] holds four
    512-pixel banks stacked at partition offsets {0,32,64,96} via
    tile_position col-tiling (the 4 matmuls run concurrently in separate
    PE column groups).  Stationary = centT [128c, 32] (19 real cols +
    13 zero cols so all 32 partitions get written), moving = feature
    [128c, 512].  Two chunk matmuls accumulate C=256.
  - scalar-engine copy fuses m = 2*dots - csq (per-partition bias) while
    moving the quad PSUM->SBUF
  - PE transposes of [128,128] slices flip pixels onto partitions:
    out[128px, 4 groups x 32] -> batched DVE argmax via
    reduce_max / is_ge / *(19-k) / reduce_max (first-index tie-break)
  - y = 19 - argmin flows through: PE transpose of the [128, 66] block
    matrix + DRAM bounce reshapes flat pixel order into [65, 129]
  - column nearest-upsample via segmented broadcast copies -> E [65,1024] bf16
  - row nearest-upsample via one-hot gather matmul G^T @ E (bf16) ->
    [512, 1024], converted to idx = 19 - y and int32 on the scalar engine
"""

import numpy as np

B, C, h, w = 8, 256, 65, 129
K = 19
H, W = 512, 1024
HW = h * w              # 8385
QUAD_PX = 2048          # four 512-px banks per psum quad
NFULL = HW // QUAD_PX   # 4 full quads
REM = HW - NFULL * QUAD_PX   # 193 remainder pixels
NT = (HW + 127) // 128  # 66 pixel blocks of 128 (for the block matrix)


def _col_segments():
    """Segments of the nearest-neighbor column map ci[c'] = c'*129 // 1024."""
    ci = (np.arange(W) * w) // W
    reps = np.bincount(ci, minlength=w)
    segs = []
    i, dst = 0, 0
    while i < w:
        j = i
        while j < w and reps[j] == reps[i]:
            j += 1
        segs.append((i, j - i, int(reps[i]), dst))
        dst += (j - i) * int(reps[i])
        i = j
    assert dst == W
    return segs


def _row_onehot():
    """G[s, r'] = 1.0 iff floor(r'*65/512) == s; shape [65, 512] bf16."""
    import ml_dtypes

    ri = (np.arange(H) * h) // H
    return (ri[None, :] == np.arange(h)[:, None]).astype(ml_dtypes.bfloat16)


def build_module(num_devices=8):
    import concourse.bass as bass
    import concourse.tile as tile
    from concourse import bacc, mybir

    f32 = mybir.dt.float32
    bf16 = mybir.dt.bfloat16
    i32 = mybir.dt.int32

    nc = bacc.Bacc(
        "TRN2",
        target_bir_lowering=False,
        debug=False,
        enable_asserts=False,
        num_devices=num_devices,
    )

    f_s2t = nc.dram_tensor("feature_s2t", [C, HW], f32, kind="ExternalInput")
    f_tgt = nc.dram_tensor("feature_target", [C, HW], f32, kind="ExternalInput")
    c_s2t = nc.dram_tensor("centroid_s2t", [K, C], f32, kind="ExternalInput")
    c_tgt = nc.dram_tensor("centroid_target", [K, C], f32, kind="ExternalInput")
    out0 = nc.dram_tensor("out0", [H, W], i32, kind="ExternalOutput")
    out1 = nc.dram_tensor("out1", [H, W], i32, kind="ExternalOutput")

    ident_dram = nc.inline_tensor(np.eye(128, dtype=np.float32), name="ident_const")
    g_dram = nc.inline_tensor(_row_onehot(), name="rowgather_const")
    wk_np = np.tile((K - np.arange(K)).astype(np.float32), (128, 1))
    wk_dram = nc.inline_tensor(wk_np, name="wk_const")
    # sel[k, 32j+k] = -1.0: replicates -csq over the four 32-partition groups
    sel_np = np.zeros((K, 128), dtype=np.float32)
    for j in range(4):
        sel_np[np.arange(K), 32 * j + np.arange(K)] = -1.0
    sel_dram = nc.inline_tensor(sel_np, name="sel_const")

    col_segs = _col_segments()
    X = mybir.AxisListType.X
    ALU = mybir.AluOpType
    AF = mybir.ActivationFunctionType

    with tile.TileContext(nc) as tc:
        from contextlib import ExitStack

        with ExitStack() as ctx:
            const_p = ctx.enter_context(tc.tile_pool(name="const", bufs=1))
            feat_p = ctx.enter_context(tc.tile_pool(name="feat", bufs=4))
            q_p = ctx.enter_context(tc.tile_pool(name="q", bufs=3))
            s_p = ctx.enter_context(tc.tile_pool(name="s", bufs=2))
            pt_p = ctx.enter_context(tc.tile_pool(name="pt", bufs=2))
            m_p = ctx.enter_context(tc.tile_pool(name="m", bufs=2))
            oi_p = ctx.enter_context(tc.tile_pool(name="oi", bufs=3))
            ps_dist = ctx.enter_context(tc.tile_pool(name="psd", bufs=3, space="PSUM"))
            ps_tr = ctx.enter_context(tc.tile_pool(name="pst", bufs=3, space="PSUM"))
            ps_out = ctx.enter_context(tc.tile_pool(name="pso", bufs=2, space="PSUM"))
            dram_p = ctx.enter_context(tc.tile_pool(name="dram", bufs=2, space="DRAM"))

            # ---- constants (centroids first: they gate the prep chain) ----
            cent_sbs = {}
            for pidx, cdram in ((0, c_tgt), (1, c_s2t)):
                cs = const_p.tile(
                    [K, C], f32, tag=f"cent{pidx}", name=f"cent_sb{pidx}"
                )
                nc.sync.dma_start(out=cs[:], in_=cdram[:, :])
                cent_sbs[pidx] = cs
            ident = const_p.tile([128, 128], f32, tag="ident")
            nc.sync.dma_start(out=ident[:], in_=ident_dram[:, :])
            g_sb = const_p.tile([h, H], bf16, tag="gmat")
            nc.sync.dma_start(out=g_sb[:], in_=g_dram[:, :])
            wk_sb = const_p.tile([128, K], f32, tag="wk")
            nc.sync.dma_start(out=wk_sb[:], in_=wk_dram[:, :])
            sel_sb = const_p.tile([K, 128], f32, tag="sel")
            nc.sync.dma_start(out=sel_sb[:], in_=sel_dram[:, :])
            k19_sb = const_p.tile([128, 1], f32, tag="k19")
            nc.vector.memset(k19_sb[:], float(K))

            # ---- per-pair centroid prep ----
            def prep_pair(cent_dram, pidx):
                cent_sb = cent_sbs[pidx]
                sq = const_p.tile([K, C], f32, tag=f"centsq{pidx}")
                nc.vector.tensor_mul(sq[:], cent_sb[:], cent_sb[:])
                csq = const_p.tile([K, 1], f32, tag=f"csq{pidx}")
                nc.vector.reduce_sum(csq[:], sq[:], axis=X)
                # -csq replicated at partition offsets {0,32,64,96}
                pb = ps_tr.tile([128, 1], f32, tag="tr")
                nc.tensor.matmul(pb[:], sel_sb[:], csq[:], start=True, stop=True)
                csqn4 = const_p.tile([128, 1], f32, tag=f"csqn4_{pidx}")
                nc.vector.tensor_copy(out=csqn4[:], in_=pb[:])
                # centT chunks [128, 32]: cols 0:19 = cent^T, cols 19:32 = 0
                centT = []
                for cc in range(2):
                    ct = const_p.tile([128, 32], f32, tag=f"centT{pidx}_{cc}")
                    nc.vector.memset(ct[:], 0.0)
                    pt = ps_tr.tile([128, K], f32, tag="tr")
                    nc.tensor.transpose(
                        pt[:], cent_sb[:, cc * 128 : (cc + 1) * 128], ident[:K, :K]
                    )
                    nc.vector.tensor_copy(out=ct[:, 0:K], in_=pt[:])
                    centT.append(ct)
                return centT, csqn4

            centT_tgt, csqn4_tgt = prep_pair(c_tgt, 0)   # for mask0 (feature_s2t)
            centT_s2t, csqn4_s2t = prep_pair(c_s2t, 1)   # for mask1 (feature_target)

            # ---- per-mask pipeline, phase-sorted emission ----
            # Phase A (stream): per mask, feature loads + dist quads + PE
            # transposes + inline DVE argmin pieces.  Nothing here waits on
            # the upsample chain, so the per-engine FIFOs never stall.
            # Phase B (finish): block-transposes + DRAM bounce + column
            # upsample + gather/convert/store for both masks, interleaved.

            class MaskCtx:
                pass

            def stream_mask(feat, centT, csqn4, out_dram, sprinkle=()):
                mc = MaskCtx()
                mc.out_dram = out_dram
                # sg layout: value for pixel block b (= p//128), class k at
                # column 19*b + k  (66 blocks x 19 = 1254, padded)
                mc.sg = s_p.tile([128, NT * K + 40], f32, tag="s")
                mc.ptf = pt_p.tile([128, NT], f32, tag="ptf")
                mc.mx = pt_p.tile([128, NT], f32, tag="mx")
                mc.eq = s_p.tile([128, NT * K], f32, tag="eq")
                mc.scratch = dram_p.tile([NT, 128], f32, tag="scratch")
                mc.msb = m_p.tile([h, w], f32, tag="m")
                mc.e_sb = m_p.tile([h, W], bf16, tag="e")
                # rows may be read (x0 in G) by gather chunks before the
                # later colexp parts write them — keep them initialized
                nc.gpsimd.memset(mc.e_sb[:], 0.0)

                def argmin_piece(b0, b1):
                    # y = 19 - argmin over k, first-index tie-break
                    nb = b1 - b0
                    sl = mc.sg[:, K * b0 : K * b1].rearrange("p (b k) -> p b k", k=K)
                    mxs = mc.mx[:, b0:b1]
                    nc.vector.tensor_reduce(mxs, sl, axis=X, op=ALU.max)
                    eqs = mc.eq[:, K * b0 : K * b1].rearrange(
                        "p (b k) -> p b k", k=K
                    )
                    nc.vector.tensor_tensor(
                        out=eqs,
                        in0=sl,
                        in1=mxs.unsqueeze(2).broadcast_to([128, nb, K]),
                        op=ALU.is_ge,
                    )
                    nc.vector.tensor_tensor(
                        out=eqs,
                        in0=eqs,
                        in1=wk_sb[:].unsqueeze(1).broadcast_to([128, nb, K]),
                        op=ALU.mult,
                    )
                    nc.vector.tensor_reduce(
                        mc.ptf[:, b0:b1], eqs, axis=X, op=ALU.max
                    )

                def load_range(px0, pxw, fine=False):
                    fg = []
                    for cc in range(2):
                        ft = feat_p.tile([128, QUAD_PX], f32, tag=f"feat{cc}")
                        half = (pxw + 1) // 2
                        nc.sync.dma_start(
                            out=ft[:, 0:half],
                            in_=feat[cc * 128 : (cc + 1) * 128, px0 : px0 + half],
                        )
                        nc.sync.dma_start(
                            out=ft[:, half:pxw],
                            in_=feat[cc * 128 : (cc + 1) * 128, px0 + half : px0 + pxw],
                        )
                        fg.append(ft)
                    return fg

                def do_quad(Bq, fine=False):
                    # full quad: 4 col-groups x 2 chunks
                    fg = load_range(Bq * QUAD_PX, QUAD_PX, fine=fine)
                    psq = ps_dist.tile([128, 512], f32, tag="dist")
                    for j in range(4):
                        for cc in range(2):
                            nc.tensor.matmul(
                                psq[32 * j : 32 * j + 32, :],
                                centT[cc][:],
                                fg[cc][:, 512 * j : 512 * j + 512],
                                start=(cc == 0),
                                stop=(cc == 1),
                                tile_position=(0, 32 * j),
                            )
                    quad = q_p.tile([128, 512], f32, tag="quad")
                    nc.scalar.activation(
                        out=quad[:],
                        in_=psq[:],
                        func=AF.Identity,
                        bias=csqn4[:],
                        scale=2.0,
                    )
                    ptr4 = ps_tr.tile([128, 512], f32, tag="tr")
                    for tq in range(4):
                        nc.tensor.transpose(
                            ptr4[:, 128 * tq : 128 * tq + 128],
                            quad[:, 128 * tq : 128 * tq + 128],
                            ident[:],
                        )
                    # ptr4 col = 128*tq + 32*j + k'; block b = 16*Bq + 4*j + tq
                    base = K * 16 * Bq
                    nc.vector.tensor_copy(
                        out=mc.sg[:, base : base + 16 * K].rearrange(
                            "p (j tq k) -> p j tq k", tq=4, k=K
                        ),
                        in_=ptr4[:]
                        .rearrange("p (tq j e) -> p tq j e", j=4, e=32)[:, :, :, 0:K]
                        .transpose([0, 2, 1, 3]),
                    )

                def do_rem():
                    # remainder: 193 px, single group
                    px0 = NFULL * QUAD_PX
                    pxw = HW - px0
                    fg = load_range(px0, pxw)
                    psr = ps_dist.tile([32, 256], f32, tag="dist")
                    nc.vector.memset(psr[:, pxw:256], 0.0)
                    for cc in range(2):
                        nc.tensor.matmul(
                            psr[0:32, 0:pxw],
                            centT[cc][:],
                            fg[cc][:, 0:pxw],
                            start=(cc == 0),
                            stop=(cc == 1),
                        )
                    st2 = q_p.tile([32, 256], f32, tag="st2")
                    nc.scalar.activation(
                        out=st2[:],
                        in_=psr[:],
                        func=AF.Identity,
                        bias=csqn4[0:32, :],
                        scale=2.0,
                    )
                    for tq in range(2):
                        b = 64 + tq
                        ptr = ps_tr.tile([128, 32], f32, tag="tr")
                        nc.tensor.transpose(
                            ptr[:], st2[:, 128 * tq : 128 * tq + 128], ident[:32, :32]
                        )
                        nc.vector.tensor_copy(
                            out=mc.sg[:, K * b : K * b + K],
                            in_=ptr[:, 0:K],
                        )

                sp = list(sprinkle) + [None] * 5

                def run_sp(i):
                    if sp[i]:
                        sp[i](mc)

                do_quad(0)
                run_sp(0)
                do_quad(1)
                run_sp(1)
                do_quad(2)
                argmin_piece(0, 48)
                run_sp(2)
                do_rem()
                argmin_piece(64, NT)
                run_sp(3)
                do_quad(3)
                argmin_piece(48, 64)
                run_sp(4)
                return mc

            def ptt_piece(mc, b0, b1):
                # block matrix -> flat pixel order, piecewise
                nb = b1 - b0
                ptt = ps_tr.tile([nb, 128], f32, tag="tr")
                nc.tensor.transpose(ptt[:], mc.ptf[:, b0:b1], ident[:])
                pttsb = pt_p.tile([nb, 128], f32, tag="pttsb")
                nc.vector.tensor_copy(out=pttsb[:], in_=ptt[:])
                nc.scalar.dma_start(out=mc.scratch[b0:b1, :], in_=pttsb[:])

            def m_dma(mc, r0, r1):
                nc.gpsimd.dma_start(
                    out=mc.msb[r0:r1, :],
                    in_=mc.scratch[:]
                    .rearrange("a b -> (a b)")[r0 * w : r1 * w]
                    .rearrange("(r c) -> r c", c=w),
                )

            def colexp(mc, r0, r1):
                # column nearest-upsample 129 -> 1024 on rows [r0:r1]
                for src0, nsrc, rep, dst0 in _col_segments():
                    nc.vector.tensor_copy(
                        out=mc.e_sb[r0:r1, dst0 : dst0 + nsrc * rep].rearrange(
                            "p (s r) -> p s r", r=rep
                        ),
                        in_=mc.msb[r0:r1, src0 : src0 + nsrc]
                        .unsqueeze(2)
                        .broadcast_to([r1 - r0, nsrc, rep]),
                    )

            def gather_chunk(mc, n):
                # row nearest-upsample rows [128n, 128n+128) + int convert
                for hh in range(W // 512):
                    po = ps_out.tile([128, 512], f32, tag="out")
                    nc.tensor.matmul(
                        po[:],
                        g_sb[:, n * 128 : (n + 1) * 128],
                        mc.e_sb[:, hh * 512 : (hh + 1) * 512],
                        start=True,
                        stop=True,
                    )
                    oint = oi_p.tile([128, 512], i32, tag="oint")
                    # idx = 19 - y, cast to int32
                    nc.scalar.activation(
                        out=oint[:],
                        in_=po[:],
                        func=AF.Copy,
                        bias=float(K),
                        scale=-1.0,
                    )
                    nc.gpsimd.dma_start(
                        out=mc.out_dram[
                            n * 128 : (n + 1) * 128, hh * 512 : (hh + 1) * 512
                        ],
                        in_=oint[:],
                    )

            mc0 = stream_mask(f_s2t, centT_tgt, csqn4_tgt, out0)

            def finish_pieces(mc):
                ptt_piece(mc, 0, 48)
                ptt_piece(mc, 64, NT)
                ptt_piece(mc, 48, 64)
                m_dma(mc, 0, 33)
                m_dma(mc, 64, h)
                m_dma(mc, 32, 64)

            mc1 = stream_mask(
                f_tgt,
                centT_s2t,
                csqn4_s2t,
                out1,
                sprinkle=(
                    lambda _mc: finish_pieces(mc0),
                    lambda _mc: colexp(mc0, 0, 33),
                    lambda _mc: (colexp(mc0, 32, 64), colexp(mc0, 64, h)),
                    lambda _mc: gather_chunk(mc0, 0),
                    lambda _mc: (
                        gather_chunk(mc0, 1),
                        gather_chunk(mc0, 2),
                        gather_chunk(mc0, 3),
                        finish_pieces(_mc),
                        colexp(_mc, 0, 33),
                        colexp(_mc, 64, h),
                    ),
                ),
            )

            gather_chunk(mc1, 0)
            gather_chunk(mc1, 1)
            colexp(mc1, 32, 64)
            gather_chunk(mc1, 2)
            gather_chunk(mc1, 3)

    nc.compile()
    return nc


_cached_nc = None


def _get_nc():
    global _cached_nc
    if _cached_nc is None:
        _cached_nc = build_module()
    return _cached_nc


def make_in_maps(feature_s2t, feature_target, centroid_s2t, centroid_target):
    in_maps = []
    for b in range(B):
        in_maps.append(
            {
                "feature_s2t": np.ascontiguousarray(
                    feature_s2t[b], dtype=np.float32
                ).reshape(C, HW),
                "feature_target": np.ascontiguousarray(
                    feature_target[b], dtype=np.float32
                ).reshape(C, HW),
                "centroid_s2t": np.ascontiguousarray(centroid_s2t, dtype=np.float32),
                "centroid_target": np.ascontiguousarray(
                    centroid_target, dtype=np.float32
                ),
            }
        )
    return in_maps


def kernel(
    feature_s2t,
    feature_target,
    centroid_s2t,
    centroid_target,
    seg_s2t=None,
    seg_target=None,
    **_unused,
):
    from concourse.bass_utils import run_bass_kernel_spmd

    nc = _get_nc()
    in_maps = make_in_maps(
        np.asarray(feature_s2t),
        np.asarray(feature_target),
        np.asarray(centroid_s2t),
        np.asarray(centroid_target),
    )
    res = run_bass_kernel_spmd(nc, in_maps, core_ids=list(range(B)))
    results = res.results
    m0 = np.stack([results[b]["out0"] for b in range(B)]).astype(np.int32)
    m1 = np.stack([results[b]["out1"] for b in range(B)]).astype(np.int32)
    return (m0, m1)
